# revision 19
# baseline (speedup 1.0000x reference)
"""Trainium2 Bass kernel for nn_CombinedLoss (LCCNet CombinedLoss).

Strategy (final)
----------------
Heavy part: loss_pc = sum_b mean_n ||(RT_inv_b - I) @ p_n||. Per sample the
displacement is d = A3 q + a4 (q = xyz). Column-pivoted QR A3 = Q R gives
    err^2 = (s1*(u1 + a*u2 + b*u3) + b1)^2 + (s2*(u2 + g*u3) + b2)^2 + b3^2
with bounded ratios a,b,g (fp16-safe). Pose loss + 4x4 algebra: host float64.

Device mapping (8 cores, data-parallel over batch; 4 samples per core):
  - Host stages a per-core [128, 3, 6250] fp16 array: partitions 0-31 are
    sample 0's 32 lanes (6250 points each), etc. Per-sample constants become
    per-PARTITION [128,1] scalars, so one instruction covers all 4 samples at
    full width: tensor_scalar runs in 4x DVE perf mode and tensor_tensor in
    2x (the 1x scalar_tensor_tensor of the earlier version is avoided
    entirely). 32*6250 = 200000, so there is no host tail. fp16 staging
    halves device HBM reads; ~1563-point chunks keep DMA descriptors at the
    empirically fastest ~3.1KB size (one SWDGE queue; multi-queue splits
    measured slower due to first-chunk starvation / small descriptors).
  - 5 free-dim chunks, small first chunk (fast pipeline fill) and small last
    chunk (short drain), software-pipelined across DVE and ACT:
      DVE per chunk: m2a = a*u2, m3b = b*u3, m3g = g*u3 (tensor_scalar, 4x);
          v1 = (u1 + m2a) + m3b, v2 = u2 + m3g, X = t1 + t2 (tensor_tensor,
          2x). X(c) is interleaved between later chunks' groups.
      ACT per chunk: t1 = Square(s1*v1 + b1), t2 = Square(s2*v2 + b2),
          e = Sqrt(X + b3^2) with accum_out -> acc[:, c] (free reduction).
      SP: consts load (so Pool's first DMA is chunk 0) + result store.
  - An ACT warmup op pulls the lazy activation-table load into the initial
    DMA wait, off the critical path.
All cross-engine RAW hazards are semaphore-ordered; same-engine RAW pairs
always have >= 1 intervening op (covers the SBUF write-ack latency).
Host combine: per-sample partition sums in float64, pose loss, total.
"""

import numpy as np

B = 32
N = 200000
NCORES = 8
SPC = B // NCORES          # samples per core
NPART = 128
LANES = 32                 # partitions per sample
FREE = N // LANES          # 6250 points per partition
CHUNKS = (625, 1562, 1563, 1563, 937)
NCH = len(CHUNKS)

_CACHED_NC = None


def _quat_to_rot(q):
    """Normalized quaternion [w,x,y,z] -> 3x3 rotation matrix (float64)."""
    q = q / np.linalg.norm(q)
    w, x, y, z = q
    return np.array([
        [1 - 2*y*y - 2*z*z, 2*x*y - 2*z*w,     2*x*z + 2*y*w],
        [2*x*y + 2*z*w,     1 - 2*x*x - 2*z*z, 2*y*z - 2*x*w],
        [2*x*z - 2*y*w,     2*y*z + 2*x*w,     1 - 2*x*x - 2*y*y],
    ])


def _pivoted_qr(A3):
    """Column-pivoted QR of a 3x3 matrix (float64). A3[:, piv] = Q @ R.

    Modified Gram-Schmidt with greedy max-residual-norm pivoting, which
    guarantees |R[i, j]| <= |R[i, i]| for j > i (bounded ratios)."""
    cols = {c: A3[:, c].astype(np.float64).copy() for c in range(3)}
    coeff = {c: np.zeros(3) for c in range(3)}   # coeff[c][i] = Q[:,i].A3[:,c]
    remaining = [0, 1, 2]
    piv = []
    Q = np.zeros((3, 3))
    for i in range(3):
        cbest = max(remaining, key=lambda c: float(np.dot(cols[c], cols[c])))
        remaining.remove(cbest)
        piv.append(cbest)
        v = cols[cbest]
        nrm = np.sqrt(np.dot(v, v))
        if nrm < 1e-300:
            # Degenerate column: pick any unit vector orthogonal to prior qs.
            for basis in np.eye(3):
                w = basis - Q[:, :i] @ (Q[:, :i].T @ basis)
                if np.dot(w, w) > 1e-12:
                    v = w
                    break
            nrm = np.sqrt(np.dot(v, v))
        q = v / nrm
        Q[:, i] = q
        for c in [cbest] + remaining:
            proj = float(np.dot(q, cols[c]))
            coeff[c][i] = proj
            cols[c] = cols[c] - proj * q
    R = np.stack([coeff[c] for c in piv], axis=1)
    return Q, R, piv


def _per_sample_host(tt, tr, te, re_):
    """Returns (piv, consts[8] float32) for one sample."""
    R_t = _quat_to_rot(tr.astype(np.float64))
    R_p = _quat_to_rot(re_.astype(np.float64))
    A3 = R_p.T @ R_t - np.eye(3)
    a4 = R_p.T @ (tt.astype(np.float64) - te.astype(np.float64))
    Q, R, piv = _pivoted_qr(A3)
    b4 = Q.T @ a4
    r11, r12, r13 = R[0, 0], R[0, 1], R[0, 2]
    r22, r23 = R[1, 1], R[1, 2]
    alpha = r12 / r11 if abs(r11) > 1e-30 else 0.0
    beta = r13 / r11 if abs(r11) > 1e-30 else 0.0
    gamma = r23 / r22 if abs(r22) > 1e-30 else 0.0
    # device const layout: [a, b, g, s1, b1, s2, b2, b3^2]
    consts = np.array([alpha, beta, gamma, r11, b4[0], r22, b4[1], b4[2] ** 2])
    return piv, consts


def _build_nc():
    import concourse.bass as bass
    from concourse import mybir
    from contextlib import ExitStack

    f16, f32 = mybir.dt.float16, mybir.dt.float32
    Alu = mybir.AluOpType
    Act = mybir.ActivationFunctionType

    nc = bass.Bass("TRN2", target_bir_lowering=False, debug=False,
                   num_devices=NCORES)
    pck = nc.dram_tensor("pck", [NPART, 3, FREE], f16, kind="ExternalInput").ap()
    ctd = nc.dram_tensor("ct", [NPART, 8], f32, kind="ExternalInput").ap()
    acc_out = nc.dram_tensor("acc", [NPART, NCH], f32, kind="ExternalOutput").ap()

    offs = np.cumsum((0,) + CHUNKS)

    with ExitStack() as ctx:
        E = ctx.enter_context
        U = E(nc.sbuf_tensor("U", [NPART, 3, FREE], f16))
        cts = E(nc.sbuf_tensor("cts", [NPART, 8], f32))
        acc = E(nc.sbuf_tensor("acc_sb", [NPART, NCH], f32))
        warm = E(nc.sbuf_tensor("warm", [NPART, 1], f32))

        def tiles(nm):
            return [E(nc.sbuf_tensor(f"{nm}{c}", [NPART, CHUNKS[c]], f16))
                    for c in range(NCH)]

        m2a, m3b, m3g = tiles("m2a"), tiles("m3b"), tiles("m3g")
        v1a, v1s, v2s = tiles("v1a"), tiles("v1"), tiles("v2")
        t1s, t2s, xs, es = tiles("t1"), tiles("t2"), tiles("x"), tiles("e")

        sem_c = E(nc.semaphore("sem_c"))
        sem_u = [E(nc.semaphore(f"sem_u{c}")) for c in range(NCH)]
        sem_dve = E(nc.semaphore("sem_dve"))
        sem_act = E(nc.semaphore("sem_act"))
        sem_out = E(nc.semaphore("sem_out"))
        block = E(nc.Block())

        def cst(i):
            return cts[:, i:i + 1]

        # Interleaved schedules: [A0, A1, B0, A2, B1, ..., B(L-2), B(L-1)];
        # tick = position in that engine's inc order.
        def sched(L):
            order = [("A", 0), ("A", 1)] if L > 1 else [("A", 0)]
            for i in range(L - 2):
                order.append(("B", i))
                order.append(("A", i + 2))
            if L > 1:
                order.append(("B", L - 2))
            order.append(("B", L - 1))
            tick_a, tick_b = {}, {}
            for pos, (kind, i) in enumerate(order):
                (tick_a if kind == "A" else tick_b)[i] = pos + 1
            return order, tick_a, tick_b

        order, G_tick, X_tick = sched(NCH)   # DVE: A=G group, B=X add
        _, T_tick, S_tick = sched(NCH)       # ACT: A=T squares, B=S sqrt

        @block.gpsimd
        def _(g):
            for c in range(NCH):
                o, w = offs[c], CHUNKS[c]
                g.dma_start(U[:, :, o:o + w], pck[:, :, o:o + w]) \
                    .then_inc(sem_u[c], 16)



        @block.vector
        def _(v):
            v.wait_ge(sem_c, 16)

            def G(c):
                o, w = offs[c], CHUNKS[c]
                u1 = U[:, 0, o:o + w]
                u2 = U[:, 1, o:o + w]
                u3 = U[:, 2, o:o + w]
                v.wait_ge(sem_u[c], 16)
                v.tensor_scalar_mul(m2a[c][:], u2, cst(0))
                v.tensor_scalar_mul(m3b[c][:], u3, cst(1))
                v.tensor_scalar_mul(m3g[c][:], u3, cst(2))
                v.tensor_tensor(v1a[c][:], u1, m2a[c][:], Alu.add)
                v.tensor_tensor(v2s[c][:], u2, m3g[c][:], Alu.add)
                v.tensor_tensor(v1s[c][:], v1a[c][:], m3b[c][:], Alu.add) \
                    .then_inc(sem_dve, 1)

            def X(c):
                v.wait_ge(sem_act, T_tick[c])
                v.tensor_tensor(xs[c][:], t1s[c][:], t2s[c][:], Alu.add) \
                    .then_inc(sem_dve, 1)

            for kind, i in order:
                (G if kind == "A" else X)(i)

        @block.scalar
        def _(s):
            s.wait_ge(sem_c, 16)
            # Warmup: trigger the lazy activation-table load while the first
            # point-cloud chunk is still in flight (off the critical path).
            s.activation(warm[:], cts[:, 0:1], Act.Square)

            def T(c):
                s.wait_ge(sem_dve, G_tick[c])
                s.activation(t1s[c][:], v1s[c][:], Act.Square,
                             bias=cst(4), scale=cst(3))
                s.activation(t2s[c][:], v2s[c][:], Act.Square,
                             bias=cst(6), scale=cst(5)).then_inc(sem_act, 1)

            def S(c):
                s.wait_ge(sem_dve, X_tick[c])
                s.activation(es[c][:], xs[c][:], Act.Sqrt, bias=cst(7),
                             accum_out=acc[:, c:c + 1]).then_inc(sem_act, 1)

            for kind, i in order:
                (T if kind == "A" else S)(i)

        @block.sync
        def _(sp):
            # Consts load from the (otherwise idle) sync engine so the Pool
            # engine's first DMA_DIRECT2D is point-cloud chunk 0; the odd
            # chunks ride SP's separate HWDGE queue in parallel with Pool's.
            sp.dma_start(cts[:], ctd).then_inc(sem_c, 16)
            sp.wait_ge(sem_act, 2 * NCH)
            sp.dma_start(acc_out, acc[:]).then_inc(sem_out, 16)
            sp.wait_ge(sem_out, 16)

    return nc


def _get_nc():
    global _CACHED_NC
    if _CACHED_NC is None:
        _CACHED_NC = _build_nc()
    return _CACHED_NC


def _kernel_impl(point_clouds, target_transl, target_rot, transl_err, rot_err,
                 trace=False):
    from concourse.bass_utils import run_bass_kernel_spmd

    pc = np.asarray(point_clouds)
    tt = np.asarray(target_transl, np.float64)
    tr = np.asarray(target_rot, np.float64)
    te = np.asarray(transl_err, np.float64)
    re_ = np.asarray(rot_err, np.float64)

    # ---- pose loss (host, float64, exact reference formulas) ----
    d = np.abs(te - tt)
    loss_transl = np.where(d < 1.0, 0.5 * d * d, d - 0.5).sum(axis=1).mean()

    rinv = tr * np.array([1.0, -1.0, -1.0, -1.0])
    q = re_
    w = q[:, 0]*rinv[:, 0] - q[:, 1]*rinv[:, 1] - q[:, 2]*rinv[:, 2] - q[:, 3]*rinv[:, 3]
    x = q[:, 0]*rinv[:, 1] + q[:, 1]*rinv[:, 0] + q[:, 2]*rinv[:, 3] - q[:, 3]*rinv[:, 2]
    y = q[:, 0]*rinv[:, 2] - q[:, 1]*rinv[:, 3] + q[:, 2]*rinv[:, 0] + q[:, 3]*rinv[:, 1]
    z = q[:, 0]*rinv[:, 3] + q[:, 1]*rinv[:, 2] - q[:, 2]*rinv[:, 1] + q[:, 3]*rinv[:, 0]
    angle = 2.0 * np.arctan2(np.sqrt(x*x + y*y + z*z), np.abs(w))
    loss_rot = (180.0 * angle / np.pi).mean()
    pose_loss = loss_transl + loss_rot

    # ---- per-sample transform constants (host) ----
    all_consts = np.zeros((B, 8))
    all_piv = []
    for b in range(B):
        piv, consts = _per_sample_host(tt[b], tr[b], te[b], re_[b])
        all_consts[b] = consts
        all_piv.append(piv)

    # ---- build per-core staged inputs (fp16, partition-major layout) ----
    in_maps = []
    for k in range(NCORES):
        blocks = []
        crows = []
        for j in range(SPC):
            b = k * SPC + j
            arr = pc[b][all_piv[b], :].astype(np.float16)     # [3, 200000]
            blocks.append(arr.reshape(3, LANES, FREE).transpose(1, 0, 2))
            crows.append(np.repeat(all_consts[b][None, :], LANES, axis=0))
        in_maps.append({
            "pck": np.ascontiguousarray(np.concatenate(blocks, axis=0)),
            "ct": np.concatenate(crows, axis=0).astype(np.float32),
        })

    nc = _get_nc()
    res = run_bass_kernel_spmd(nc, in_maps, core_ids=list(range(NCORES)),
                               trace=trace)

    # ---- combine (host, float64) ----
    pcl_sum = 0.0
    for k in range(NCORES):
        accm = res.results[k]["acc"].astype(np.float64)   # [128, NCH]
        for j in range(SPC):
            pcl_sum += accm[j * LANES:(j + 1) * LANES, :].sum() / N

    total = 0.5 * pose_loss + 0.5 * (pcl_sum / B)
    out = (np.float32(total), np.float32(loss_transl), np.float32(loss_rot),
           np.float32(pcl_sum / B))
    return out, res


def kernel(point_clouds, target_transl, target_rot, transl_err, rot_err):
    out, _ = _kernel_impl(point_clouds, target_transl, target_rot,
                          transl_err, rot_err)
    return out


# revision 21
# speedup vs baseline: 1.0419x; 1.0419x over previous
"""Trainium2 Bass kernel for nn_CombinedLoss (LCCNet CombinedLoss).

Strategy (final)
----------------
Heavy part: loss_pc = sum_b mean_n ||(RT_inv_b - I) @ p_n||. Per sample the
displacement is d = A3 q + a4 (q = xyz). Column-pivoted QR A3 = Q R gives
    err^2 = (s1*(u1 + a*u2 + b*u3) + b1)^2 + (s2*(u2 + g*u3) + b2)^2 + b3^2
with bounded ratios a,b,g (fp16-safe). Pose loss + 4x4 algebra: host float64.

Device mapping (8 cores, data-parallel over batch; 4 samples per core):
  - Host stages a per-core [128, 3, 6250] fp16 array: partitions 0-31 are
    sample 0's 32 lanes (6250 points each), etc. Per-sample constants become
    per-PARTITION [128,1] scalars, so one instruction covers all 4 samples at
    full width: tensor_scalar runs in 4x DVE perf mode and tensor_tensor in
    2x (the 1x scalar_tensor_tensor of the earlier version is avoided
    entirely). 32*6250 = 200000, so there is no host tail. fp16 staging
    halves device HBM reads; ~1563-point chunks keep DMA descriptors at the
    empirically fastest ~3.1KB size (one SWDGE queue; multi-queue splits
    measured slower due to first-chunk starvation / small descriptors).
  - 5 free-dim chunks, small first chunk (fast pipeline fill) and small last
    chunk (short drain), software-pipelined across DVE and ACT:
      DVE per chunk: m2a = a*u2, m3b = b*u3, m3g = g*u3 (tensor_scalar, 4x);
          v1 = (u1 + m2a) + m3b, v2 = u2 + m3g, X = t1 + t2 (tensor_tensor,
          2x). X(c) is interleaved between later chunks' groups.
      ACT per chunk: t1 = Square(s1*v1 + b1), t2 = Square(s2*v2 + b2),
          e = Sqrt(X + b3^2) with accum_out -> acc[:, c] (free reduction).
      SP: consts load (so Pool's first DMA is chunk 0) + result store.
  - An ACT warmup op pulls the lazy activation-table load into the initial
    DMA wait, off the critical path.
All cross-engine RAW hazards are semaphore-ordered; same-engine RAW pairs
always have >= 1 intervening op (covers the SBUF write-ack latency).
Host combine: per-sample partition sums in float64, pose loss, total.
"""

import numpy as np

B = 32
N = 200000
NCORES = 8
SPC = B // NCORES          # samples per core
NPART = 128
LANES = 32                 # partitions per sample
FREE = N // LANES          # 6250 points per partition
CHUNKS = (625, 1562, 1563, 1563, 937)
NCH = len(CHUNKS)

_CACHED_NC = None


def _quat_to_rot(q):
    """Normalized quaternion [w,x,y,z] -> 3x3 rotation matrix (float64)."""
    q = q / np.linalg.norm(q)
    w, x, y, z = q
    return np.array([
        [1 - 2*y*y - 2*z*z, 2*x*y - 2*z*w,     2*x*z + 2*y*w],
        [2*x*y + 2*z*w,     1 - 2*x*x - 2*z*z, 2*y*z - 2*x*w],
        [2*x*z - 2*y*w,     2*y*z + 2*x*w,     1 - 2*x*x - 2*y*y],
    ])


def _pivoted_qr(A3):
    """Column-pivoted QR of a 3x3 matrix (float64). A3[:, piv] = Q @ R.

    Modified Gram-Schmidt with greedy max-residual-norm pivoting, which
    guarantees |R[i, j]| <= |R[i, i]| for j > i (bounded ratios)."""
    cols = {c: A3[:, c].astype(np.float64).copy() for c in range(3)}
    coeff = {c: np.zeros(3) for c in range(3)}   # coeff[c][i] = Q[:,i].A3[:,c]
    remaining = [0, 1, 2]
    piv = []
    Q = np.zeros((3, 3))
    for i in range(3):
        cbest = max(remaining, key=lambda c: float(np.dot(cols[c], cols[c])))
        remaining.remove(cbest)
        piv.append(cbest)
        v = cols[cbest]
        nrm = np.sqrt(np.dot(v, v))
        if nrm < 1e-300:
            # Degenerate column: pick any unit vector orthogonal to prior qs.
            for basis in np.eye(3):
                w = basis - Q[:, :i] @ (Q[:, :i].T @ basis)
                if np.dot(w, w) > 1e-12:
                    v = w
                    break
            nrm = np.sqrt(np.dot(v, v))
        q = v / nrm
        Q[:, i] = q
        for c in [cbest] + remaining:
            proj = float(np.dot(q, cols[c]))
            coeff[c][i] = proj
            cols[c] = cols[c] - proj * q
    R = np.stack([coeff[c] for c in piv], axis=1)
    return Q, R, piv


def _per_sample_host(tt, tr, te, re_):
    """Returns (piv, consts[8] float32) for one sample."""
    R_t = _quat_to_rot(tr.astype(np.float64))
    R_p = _quat_to_rot(re_.astype(np.float64))
    A3 = R_p.T @ R_t - np.eye(3)
    a4 = R_p.T @ (tt.astype(np.float64) - te.astype(np.float64))
    Q, R, piv = _pivoted_qr(A3)
    b4 = Q.T @ a4
    r11, r12, r13 = R[0, 0], R[0, 1], R[0, 2]
    r22, r23 = R[1, 1], R[1, 2]
    alpha = r12 / r11 if abs(r11) > 1e-30 else 0.0
    beta = r13 / r11 if abs(r11) > 1e-30 else 0.0
    gamma = r23 / r22 if abs(r22) > 1e-30 else 0.0
    # device const layout: [a, b, g, s1, b1, s2, b2, b3^2]
    consts = np.array([alpha, beta, gamma, r11, b4[0], r22, b4[1], b4[2] ** 2])
    return piv, consts


def _build_nc():
    import concourse.bass as bass
    from concourse import mybir
    from contextlib import ExitStack

    f16, f32 = mybir.dt.float16, mybir.dt.float32
    Alu = mybir.AluOpType
    Act = mybir.ActivationFunctionType

    nc = bass.Bass("TRN2", target_bir_lowering=False, debug=False,
                   num_devices=NCORES)
    pck = nc.dram_tensor("pck", [NPART, 3, FREE], f16, kind="ExternalInput").ap()
    ctd = nc.dram_tensor("ct", [NPART, 8], f32, kind="ExternalInput").ap()
    acc_out = nc.dram_tensor("acc", [NPART, NCH], f32, kind="ExternalOutput").ap()

    offs = np.cumsum((0,) + CHUNKS)

    with ExitStack() as ctx:
        E = ctx.enter_context
        U = E(nc.sbuf_tensor("U", [NPART, 3, FREE], f16))
        cts = E(nc.sbuf_tensor("cts", [NPART, 8], f32))
        acc = E(nc.sbuf_tensor("acc_sb", [NPART, NCH], f32))
        warm = E(nc.sbuf_tensor("warm", [NPART, 1], f32))

        def tiles(nm):
            return [E(nc.sbuf_tensor(f"{nm}{c}", [NPART, CHUNKS[c]], f16))
                    for c in range(NCH)]

        m2a, m3b, m3g = tiles("m2a"), tiles("m3b"), tiles("m3g")
        z24 = E(nc.sbuf_tensor("z24", [NPART, CHUNKS[NCH - 1]], f16))
        v1a, v1s, v2s = tiles("v1a"), tiles("v1"), tiles("v2")
        t1s, t2s, xs, es = tiles("t1"), tiles("t2"), tiles("x"), tiles("e")

        sem_c = E(nc.semaphore("sem_c"))
        sem_u = [E(nc.semaphore(f"sem_u{c}")) for c in range(NCH)]
        sem_dve = E(nc.semaphore("sem_dve"))
        sem_act = E(nc.semaphore("sem_act"))
        sem_out = E(nc.semaphore("sem_out"))
        block = E(nc.Block())

        def cst(i):
            return cts[:, i:i + 1]

        # Interleaved schedules: [A0, A1, B0, A2, B1, ..., B(L-2), B(L-1)];
        # tick = position in that engine's inc order.
        def sched(L):
            order = [("A", 0), ("A", 1)] if L > 1 else [("A", 0)]
            for i in range(L - 2):
                order.append(("B", i))
                order.append(("A", i + 2))
            if L > 1:
                order.append(("B", L - 2))
            order.append(("B", L - 1))
            tick_a, tick_b = {}, {}
            for pos, (kind, i) in enumerate(order):
                (tick_a if kind == "A" else tick_b)[i] = pos + 1
            return order, tick_a, tick_b

        order, G_tick, X_tick = sched(NCH)   # DVE: A=G group, B=X add
        _, T_tick, S_tick = sched(NCH)       # ACT: A=T squares, B=S sqrt

        @block.gpsimd
        def _(g):
            for c in range(NCH):
                o, w = offs[c], CHUNKS[c]
                g.dma_start(U[:, :, o:o + w], pck[:, :, o:o + w]) \
                    .then_inc(sem_u[c], 16)



        @block.vector
        def _(v):
            v.wait_ge(sem_c, 16)

            def G(c):
                o, w = offs[c], CHUNKS[c]
                u1 = U[:, 0, o:o + w]
                u2 = U[:, 1, o:o + w]
                u3 = U[:, 2, o:o + w]
                v.wait_ge(sem_u[c], 16)
                v.tensor_scalar_mul(m2a[c][:], u2, cst(0))
                v.tensor_scalar_mul(m3b[c][:], u3, cst(1))
                v.tensor_scalar_mul(m3g[c][:], u3, cst(2))
                v.tensor_tensor(v1a[c][:], u1, m2a[c][:], Alu.add)
                v.tensor_tensor(v2s[c][:], u2, m3g[c][:], Alu.add)
                v.tensor_tensor(v1s[c][:], v1a[c][:], m3b[c][:], Alu.add) \
                    .then_inc(sem_dve, 1)

            def X(c):
                v.wait_ge(sem_act, T_tick[c])
                v.tensor_tensor(xs[c][:], t1s[c][:], t2s[c][:], Alu.add) \
                    .then_inc(sem_dve, 1)

            G(0)
            G(1)
            X(0)
            G(2)
            X(1)
            G(3)
            X(2)
            G(4)
            # chunk 4's second square on DVE: fills the tail stall where DVE
            # would otherwise idle waiting for ACT, and shortens ACT's tail.
            v.tensor_scalar(z24[:], v2s[NCH - 1][:], cst(5), cst(6),
                            Alu.mult, Alu.add)
            X(3)
            v.tensor_tensor(t2s[NCH - 1][:], z24[:], z24[:], Alu.mult)
            X(4)

        @block.scalar
        def _(s):
            s.wait_ge(sem_c, 16)
            # Warmup: trigger the lazy activation-table load while the first
            # point-cloud chunk is still in flight (off the critical path).
            s.activation(warm[:], cts[:, 0:1], Act.Square)

            def T(c):
                s.wait_ge(sem_dve, G_tick[c])
                if c == NCH - 1:
                    s.activation(t1s[c][:], v1s[c][:], Act.Square,
                                 bias=cst(4), scale=cst(3)) \
                        .then_inc(sem_act, 1)
                    return
                s.activation(t1s[c][:], v1s[c][:], Act.Square,
                             bias=cst(4), scale=cst(3))
                s.activation(t2s[c][:], v2s[c][:], Act.Square,
                             bias=cst(6), scale=cst(5)).then_inc(sem_act, 1)

            def S(c):
                s.wait_ge(sem_dve, X_tick[c])
                s.activation(es[c][:], xs[c][:], Act.Sqrt, bias=cst(7),
                             accum_out=acc[:, c:c + 1]).then_inc(sem_act, 1)

            for kind, i in order:
                (T if kind == "A" else S)(i)

        @block.sync
        def _(sp):
            # Consts load from the (otherwise idle) sync engine so the Pool
            # engine's first DMA_DIRECT2D is point-cloud chunk 0; the odd
            # chunks ride SP's separate HWDGE queue in parallel with Pool's.
            sp.dma_start(cts[:], ctd).then_inc(sem_c, 16)
            sp.wait_ge(sem_act, 2 * NCH)
            sp.dma_start(acc_out, acc[:]).then_inc(sem_out, 16)
            sp.wait_ge(sem_out, 16)

    return nc


def _get_nc():
    global _CACHED_NC
    if _CACHED_NC is None:
        _CACHED_NC = _build_nc()
    return _CACHED_NC


def _kernel_impl(point_clouds, target_transl, target_rot, transl_err, rot_err,
                 trace=False):
    from concourse.bass_utils import run_bass_kernel_spmd

    pc = np.asarray(point_clouds)
    tt = np.asarray(target_transl, np.float64)
    tr = np.asarray(target_rot, np.float64)
    te = np.asarray(transl_err, np.float64)
    re_ = np.asarray(rot_err, np.float64)

    # ---- pose loss (host, float64, exact reference formulas) ----
    d = np.abs(te - tt)
    loss_transl = np.where(d < 1.0, 0.5 * d * d, d - 0.5).sum(axis=1).mean()

    rinv = tr * np.array([1.0, -1.0, -1.0, -1.0])
    q = re_
    w = q[:, 0]*rinv[:, 0] - q[:, 1]*rinv[:, 1] - q[:, 2]*rinv[:, 2] - q[:, 3]*rinv[:, 3]
    x = q[:, 0]*rinv[:, 1] + q[:, 1]*rinv[:, 0] + q[:, 2]*rinv[:, 3] - q[:, 3]*rinv[:, 2]
    y = q[:, 0]*rinv[:, 2] - q[:, 1]*rinv[:, 3] + q[:, 2]*rinv[:, 0] + q[:, 3]*rinv[:, 1]
    z = q[:, 0]*rinv[:, 3] + q[:, 1]*rinv[:, 2] - q[:, 2]*rinv[:, 1] + q[:, 3]*rinv[:, 0]
    angle = 2.0 * np.arctan2(np.sqrt(x*x + y*y + z*z), np.abs(w))
    loss_rot = (180.0 * angle / np.pi).mean()
    pose_loss = loss_transl + loss_rot

    # ---- per-sample transform constants (host) ----
    all_consts = np.zeros((B, 8))
    all_piv = []
    for b in range(B):
        piv, consts = _per_sample_host(tt[b], tr[b], te[b], re_[b])
        all_consts[b] = consts
        all_piv.append(piv)

    # ---- build per-core staged inputs (fp16, partition-major layout) ----
    in_maps = []
    for k in range(NCORES):
        blocks = []
        crows = []
        for j in range(SPC):
            b = k * SPC + j
            arr = pc[b][all_piv[b], :].astype(np.float16)     # [3, 200000]
            blocks.append(arr.reshape(3, LANES, FREE).transpose(1, 0, 2))
            crows.append(np.repeat(all_consts[b][None, :], LANES, axis=0))
        in_maps.append({
            "pck": np.ascontiguousarray(np.concatenate(blocks, axis=0)),
            "ct": np.concatenate(crows, axis=0).astype(np.float32),
        })

    nc = _get_nc()
    res = run_bass_kernel_spmd(nc, in_maps, core_ids=list(range(NCORES)),
                               trace=trace)

    # ---- combine (host, float64) ----
    pcl_sum = 0.0
    for k in range(NCORES):
        accm = res.results[k]["acc"].astype(np.float64)   # [128, NCH]
        for j in range(SPC):
            pcl_sum += accm[j * LANES:(j + 1) * LANES, :].sum() / N

    total = 0.5 * pose_loss + 0.5 * (pcl_sum / B)
    out = (np.float32(total), np.float32(loss_transl), np.float32(loss_rot),
           np.float32(pcl_sum / B))
    return out, res


def kernel(point_clouds, target_transl, target_rot, transl_err, rot_err):
    out, _ = _kernel_impl(point_clouds, target_transl, target_rot,
                          transl_err, rot_err)
    return out
